# revision 3
# baseline (speedup 1.0000x reference)
"""nn_ActorCopy kernel for trn2 NeuronCores (axon/PJRT execution).

Batch=1 sequential seq2seq actor with copy mechanism. The 64-step decode
loop is inherently sequential (argmax feedback through the embedding), so
the implementation keeps all weights device-resident and drives the loop
from the host: one jitted encoder call plus one jitted decode-step function
called 64 times. This avoids the pathological on-device scan schedule that
re-stages the whole loop state each iteration.
"""
import numpy as np
import jax
import jax.numpy as jnp
from jax import lax

VOCAB = 32000
EMBED = 1024
HIDDEN = 1024
ML = 64
L = 50
HH = HIDDEN // 2

_f = jnp.float32


def _lstm_cell(x, h, c, Wih, Whh, bih, bhh):
    g = x @ Wih.T + h @ Whh.T + bih + bhh
    i, f, gg, o = jnp.split(g, 4)
    return jax.nn.sigmoid(o) * jnp.tanh(
        jax.nn.sigmoid(f) * c + jax.nn.sigmoid(i) * jnp.tanh(gg)
    ), jax.nn.sigmoid(f) * c + jax.nn.sigmoid(i) * jnp.tanh(gg)


def _lstm_cell2(x, h, c, Wih, Whh, bih, bhh):
    g = x @ Wih.T + h @ Whh.T + bih + bhh
    i, f, gg, o = jnp.split(g, 4)
    i = jax.nn.sigmoid(i)
    f = jax.nn.sigmoid(f)
    gg = jnp.tanh(gg)
    o = jax.nn.sigmoid(o)
    c = f * c + i * gg
    return o * jnp.tanh(c), c


@jax.jit
def _encoder(embedding, x_tokens, Wih_f, Whh_f, bih_f, bhh_f,
             Wih_b, Whh_b, bih_b, bhh_b, copy_W, copy_b):
    xemb = embedding[x_tokens]

    def enc_step(carry, xe):
        hf, cf, hb, cb = carry
        hf, cf = _lstm_cell2(xe, hf, cf, Wih_f, Whh_f, bih_f, bhh_f)
        hb, cb = _lstm_cell2(xe, hb, cb, Wih_b, Whh_b, bih_b, bhh_b)
        return (hf, cf, hb, cb), jnp.concatenate([hf, hb])

    z = jnp.zeros((HH,), xemb.dtype)
    (hf, cf, hb, cb), enc = lax.scan(enc_step, (z, z, z, z), xemb)
    enc_outs = jnp.zeros((ML, HIDDEN), xemb.dtype).at[:L].set(enc)
    h0 = jnp.concatenate([hf, hb])
    c0 = jnp.concatenate([cf, cb])
    copy_enc = jnp.tanh(enc_outs @ copy_W.T + copy_b)
    return enc_outs, h0, c0, copy_enc


@jax.jit
def _dec_step(first, h, c, prev_pc_masked_unnorm, dec_in,
              enc_outs, copy_enc, sent_pad_f, base_mask,
              allowed_mask, embedding, Wih_d, Whh_d, b_d,
              attn_W, attn_b, gen_W, gen_b):
    """One decode step.

    prev_pc_masked_unnorm: prev_probs[VOCAB:] (the copy slice of the softmax),
    not yet masked; mask is applied here using prev_word.
    Returns new state + outputs.
    """
    a = jnp.concatenate([dec_in, h])
    attw = jax.nn.softmax(a @ attn_W.T + attn_b)
    attentive = attw @ enc_outs
    pc, prev_word_f = prev_pc_masked_unnorm
    m = base_mask * (sent_pad_f != prev_word_f).astype(_f)
    pc = pc * m
    s = pc.sum()
    pc = jnp.where(s > 0, pc / jnp.where(s > 0, s, 1.0), pc)
    selective = pc @ enc_outs
    zero = jnp.zeros_like(attentive)
    attentive = jnp.where(first, zero, attentive)
    selective = jnp.where(first, zero, selective)
    h, c = _lstm_cell2(jnp.concatenate([dec_in, selective, attentive]),
                       h, c, Wih_d, Whh_d, b_d, jnp.zeros_like(b_d))
    gen = h @ gen_W.T + gen_b
    copy = copy_enc @ h
    lo = jnp.concatenate([gen, copy])
    probs = jax.nn.softmax(lo)
    dist = probs * allowed_mask
    mx = dist.max()
    all_idx = jnp.arange(VOCAB + ML, dtype=jnp.int32)
    aidx = jnp.min(jnp.where(dist >= mx, all_idx, VOCAB + ML))
    is_voc = aidx < VOCAB
    src_f = sent_pad_f[jnp.clip(aidx - VOCAB, 0, L - 1)]
    action = jnp.where(is_voc, aidx, src_f.astype(jnp.int32)).astype(jnp.int32)
    prob = dist[aidx] + jnp.where(
        is_voc, jnp.zeros((), _f), dist[jnp.clip(action, 0, VOCAB - 1)])
    dec_in_new = embedding[action]
    return (h, c, probs[VOCAB:], action.astype(_f), dec_in_new,
            prob, action)


def kernel(x_tokens, allowed_mask, embedding, Wih_f, Whh_f, bih_f, bhh_f,
           Wih_b, Whh_b, bih_b, bhh_b, Wih_d, Whh_d, bih_d, bhh_d,
           attn_W, attn_b, gen_W, gen_b, copy_W, copy_b):
    dev = jax.devices()[0]
    put = lambda a: jax.device_put(jnp.asarray(np.asarray(a), _f)
                                   if np.asarray(a).dtype != np.int32
                                   else jnp.asarray(np.asarray(a)), dev)
    x_tokens_d = put(x_tokens)
    allowed_mask_d = put(allowed_mask)
    embedding_d = put(embedding)
    gen_W_d, gen_b_d = put(gen_W), put(gen_b)
    attn_W_d, attn_b_d = put(attn_W), put(attn_b)
    Wih_d_d, Whh_d_d = put(Wih_d), put(Whh_d)
    b_d_d = put(np.asarray(bih_d) + np.asarray(bhh_d))

    enc_outs, h0, c0, copy_enc = _encoder(
        embedding_d, x_tokens_d, put(Wih_f), put(Whh_f), put(bih_f),
        put(bhh_f), put(Wih_b), put(Whh_b), put(bih_b), put(bhh_b),
        put(copy_W), put(copy_b))

    sent_pad = np.full((ML,), -1, np.int32)
    sent_pad[:L] = np.asarray(x_tokens)
    sent_pad_f = put(sent_pad.astype(np.float32))
    pos = np.arange(ML)
    base_mask = put(((pos >= 1) & (pos < L - 1)).astype(np.float32))

    h, c = h0, c0
    pc = put(np.zeros(ML, np.float32))
    prev_word_f = put(np.float32(-1.0))
    dec_in = embedding_d[0]

    states = [h0]
    probs_out, actions_out = [], []
    for t in range(ML):
        first = jnp.asarray(t == 0)
        (h, c, pc, prev_word_f, dec_in, prob, action) = _dec_step(
            first, h, c, (pc, prev_word_f), dec_in,
            enc_outs, copy_enc, sent_pad_f, base_mask,
            allowed_mask_d, embedding_d, Wih_d_d, Whh_d_d, b_d_d,
            attn_W_d, attn_b_d, gen_W_d, gen_b_d)
        states.append(h)
        probs_out.append(prob)
        actions_out.append(action)

    states = np.stack([np.asarray(s) for s in states])
    probs = np.asarray([np.asarray(p) for p in probs_out], np.float32)
    actions = np.asarray([np.asarray(a) for a in actions_out], np.int32)
    return states, probs, actions


# revision 4
# speedup vs baseline: 2.7935x; 2.7935x over previous
"""nn_ActorCopy kernel for 8 trn2 NeuronCores.

Batch=1 sequential seq2seq actor with copy mechanism. The sequential decode
loop is latency-bound; this implementation runs the full recurrence on the
neuron devices via jax/PJRT (single-program), which keeps the large gen_W
matmul on-device. Shapes are hardcoded per the problem spec.
"""
import numpy as np
import jax
import jax.numpy as jnp
from jax import lax
from functools import partial

VOCAB = 32000
EMBED = 1024
HIDDEN = 1024
ML = 64
L = 50
HH = HIDDEN // 2


def _lstm_cell(x, h, c, Wih, Whh, bih, bhh):
    g = x @ Wih.T + h @ Whh.T + bih + bhh
    i, f, gg, o = jnp.split(g, 4)
    i = jax.nn.sigmoid(i)
    f = jax.nn.sigmoid(f)
    gg = jnp.tanh(gg)
    o = jax.nn.sigmoid(o)
    c = f * c + i * gg
    return o * jnp.tanh(c), c


@partial(jax.jit, static_argnums=())
def _forward(allowed_mask, embedding, Wih_f, Whh_f, bih_f, bhh_f,
             Wih_b, Whh_b, bih_b, bhh_b, Wih_d, Whh_d, bih_d, bhh_d,
             attn_W, attn_b, gen_W, gen_b, copy_W, copy_b, x_tokens):
    xemb = embedding[x_tokens]

    def enc_step(carry, xe):
        hf, cf, hb, cb = carry
        hf, cf = _lstm_cell(xe, hf, cf, Wih_f, Whh_f, bih_f, bhh_f)
        hb, cb = _lstm_cell(xe, hb, cb, Wih_b, Whh_b, bih_b, bhh_b)
        return (hf, cf, hb, cb), jnp.concatenate([hf, hb])

    z = jnp.zeros((HH,), xemb.dtype)
    (hf, cf, hb, cb), enc = lax.scan(enc_step, (z, z, z, z), xemb)
    enc_outs = jnp.zeros((ML, HIDDEN), xemb.dtype).at[:L].set(enc)
    h0 = jnp.concatenate([hf, hb])
    c0 = jnp.concatenate([cf, cb])

    copy_enc = jnp.tanh(enc_outs @ copy_W.T + copy_b)
    pos = jnp.arange(ML)
    sent_pad = jnp.full((ML,), -1, jnp.int32).at[:L].set(x_tokens)
    sos_emb = embedding[0]

    def dec_step(carry, t):
        h, c, prev_probs, prev_word, dec_in = carry
        first = t == 0
        a = jnp.concatenate([dec_in, h])
        attw = jax.nn.softmax(a @ attn_W.T + attn_b)
        attentive = attw @ enc_outs
        pc = prev_probs[VOCAB:]
        m = ((pos >= 1) & (pos < L - 1) & (sent_pad != prev_word)).astype(pc.dtype)
        pc = pc * m
        s = pc.sum()
        pc = jnp.where(s > 0, pc / jnp.where(s > 0, s, 1.0), pc)
        selective = pc @ enc_outs
        zero = jnp.zeros_like(attentive)
        attentive = jnp.where(first, zero, attentive)
        selective = jnp.where(first, zero, selective)
        h, c = _lstm_cell(jnp.concatenate([dec_in, selective, attentive]),
                          h, c, Wih_d, Whh_d, bih_d, bhh_d)
        gen = h @ gen_W.T + gen_b
        copy = copy_enc @ h
        probs = jnp.concatenate([gen, copy])
        probs = jax.nn.softmax(probs)
        dist = probs * allowed_mask
        mx = dist.max()
        all_idx = jnp.arange(VOCAB + ML, dtype=jnp.int32)
        aidx = jnp.min(jnp.where(dist >= mx, all_idx, VOCAB + ML))
        is_voc = aidx < VOCAB
        src = sent_pad[jnp.clip(aidx - VOCAB, 0, L - 1)]
        action = jnp.where(is_voc, aidx, src).astype(jnp.int32)
        prob = dist[aidx] + jnp.where(
            is_voc, jnp.zeros((), dist.dtype), dist[jnp.clip(action, 0, VOCAB - 1)])
        new = (h, c, lax.stop_gradient(probs), action, embedding[action])
        return new, (h, prob, action)

    carry0 = (h0, c0, jnp.zeros((VOCAB + ML,), h0.dtype),
              jnp.array(-1, jnp.int32), sos_emb)
    _, (hs, probs, actions) = lax.scan(dec_step, carry0, jnp.arange(ML))
    states = jnp.concatenate([h0[None], hs])
    return states, probs, actions


def kernel(x_tokens, allowed_mask, embedding, Wih_f, Whh_f, bih_f, bhh_f,
           Wih_b, Whh_b, bih_b, bhh_b, Wih_d, Whh_d, bih_d, bhh_d,
           attn_W, attn_b, gen_W, gen_b, copy_W, copy_b):
    dev = jax.devices()[0]
    args = [allowed_mask, embedding, Wih_f, Whh_f, bih_f, bhh_f,
            Wih_b, Whh_b, bih_b, bhh_b, Wih_d, Whh_d, bih_d, bhh_d,
            attn_W, attn_b, gen_W, gen_b, copy_W, copy_b, x_tokens]
    args = [jax.device_put(np.asarray(a), dev) for a in args]
    states, probs, actions = _forward(*args)
    return (np.asarray(states), np.asarray(probs),
            np.asarray(actions).astype(np.int32))
